# revision 16
# baseline (speedup 1.0000x reference)
"""Trainium2 Bass kernel for a 2-layer GRU (B=256, S=1024, IN=4+META=4, H=256) + FC head.

Strategy (data-parallel over batch, 8 cores, 32 batch rows each):
  - All tensors on-chip live in a "transposed" layout: partition dim = 128
    hidden/gate units (chunked), free dim = batch (32), so DVE/ACT use all
    128 lanes.
  - Per GRU step: hg^T = W_hh @ h^T via 12 weight-stationary matmuls
    (6 gate-chunks x 2 K-chunks, N=32 moving cols of h^T), accumulating
    into one PSUM tile [128, 6, 32] (fp32).  Weights stream through
    LDWEIGHTS in bf16 (FWL).
  - Input projections xg = W_ih @ x (+ both biases) are computed per
    window of T steps as efficient GEMMs, evacuated PSUM->SBUF (bf16)
    on ScalarE (ACTIVATE Identity) with the bias folded per partition.
  - The two layers' scans run software-pipelined one window apart so their
    serial gate chains interleave on the engines.
  - Everything except PSUM accumulation and the gate adds is bf16.

kernel(**inputs) takes the FULL fp32 inputs, does numpy layout prep, runs
the SPMD program on 8 cores, and concatenates the per-core [32, 1] outputs.
"""

import numpy as np
import ml_dtypes
from contextlib import ExitStack

import concourse.bass as bass
import concourse.bacc as bacc
import concourse.tile as tile
import concourse.mybir as mybir
from concourse.bass_utils import run_bass_kernel_spmd

AF = mybir.ActivationFunctionType
BF16 = mybir.dt.bfloat16
F32 = mybir.dt.float32

B = 256
NCORES = 8
BL = B // NCORES  # 32 batch rows per core
S_FULL = 1024
H = 256
G = 3 * H  # 768
KIN = 8  # IN + META
NMCH = G // 128  # 6 gate chunks
NKCH = H // 128  # 2 hidden chunks


def build_program(S=S_FULL, T=64, evac_act_ratio=2):
    """Build the single-core SPMD Bass program.

    S: sequence length; T: window (steps per xg GEMM); both scans are
    emitted interleaved with scan1 lagging scan0 by one window.
    """
    assert S % T == 0 and (T * BL) % 512 == 0
    NW = S // T
    NCH = (T * BL) // 512  # 512-wide N-chunks per window GEMM
    SPC = 512 // BL  # steps per N-chunk (16)

    nc = bacc.Bacc()

    xinT_d = nc.declare_dram_parameter("xinT", [KIN, S * BL], BF16, False)
    wih0T_d = nc.declare_dram_parameter("wih0T", [KIN, G], BF16, False)
    whh0T_d = nc.declare_dram_parameter("whh0T", [128, NKCH, G], BF16, False)
    wih1T_d = nc.declare_dram_parameter("wih1T", [128, NKCH, G], BF16, False)
    whh1T_d = nc.declare_dram_parameter("whh1T", [128, NKCH, G], BF16, False)
    b0T_d = nc.declare_dram_parameter("b0T", [128, NMCH], F32, False)
    b1T_d = nc.declare_dram_parameter("b1T", [128, NMCH], F32, False)
    b0hn_d = nc.declare_dram_parameter("b0hn", [128, SPC * NKCH * BL], BF16, False)
    b1hn_d = nc.declare_dram_parameter("b1hn", [128, SPC * NKCH * BL], BF16, False)
    b0f_d = nc.declare_dram_parameter("b0f", [128, NMCH, SPC * BL], BF16, False)
    b1f_d = nc.declare_dram_parameter("b1f", [128, NMCH, SPC * BL], BF16, False)
    fcWT_d = nc.declare_dram_parameter("fcWT", [128, NKCH], BF16, False)
    fcb_d = nc.declare_dram_parameter("fcb", [BL, 1], F32, False)
    y_d = nc.declare_dram_parameter("y", [BL, 1], F32, True)

    evac_ctr = [0]

    with ExitStack() as ctx:
        tc = ctx.enter_context(tile.TileContext(nc))
        consts = ctx.enter_context(tc.tile_pool(name="consts", bufs=1))
        xinp = ctx.enter_context(tc.tile_pool(name="xinp", bufs=2))
        xgp = ctx.enter_context(tc.tile_pool(name="xgp", bufs=2 * NCH))
        h1p = ctx.enter_context(tc.tile_pool(name="h1p", bufs=2))
        gp = ctx.enter_context(tc.tile_pool(name="gp", bufs=3))
        h2p = ctx.enter_context(tc.tile_pool(name="h2p", bufs=3))
        psc = ctx.enter_context(tc.tile_pool(name="psc", bufs=2, space="PSUM"))
        psg = ctx.enter_context(tc.tile_pool(name="psg", bufs=3, space="PSUM"))

        # ---- constants ----
        whh0_sb = consts.tile([128, NKCH, G], BF16)
        nc.sync.dma_start(whh0_sb, whh0T_d[:, :, :])
        whh1_sb = consts.tile([128, NKCH, G], BF16)
        nc.sync.dma_start(whh1_sb, whh1T_d[:, :, :])
        wih1_sb = consts.tile([128, NKCH, G], BF16)
        nc.sync.dma_start(wih1_sb, wih1T_d[:, :, :])
        wih0_sb = consts.tile([KIN, G], BF16)
        nc.sync.dma_start(wih0_sb, wih0T_d[:, :])
        b0_sb = consts.tile([128, NMCH], F32)
        nc.sync.dma_start(b0_sb, b0T_d[:, :])
        b1_sb = consts.tile([128, NMCH], F32)
        nc.sync.dma_start(b1_sb, b1T_d[:, :])
        b0hn_sb = consts.tile([128, SPC, NKCH, BL], BF16)
        nc.sync.dma_start(b0hn_sb, b0hn_d[:, :].rearrange("p (s c b) -> p s c b", s=SPC, c=NKCH))
        b1hn_sb = consts.tile([128, SPC, NKCH, BL], BF16)
        nc.sync.dma_start(b1hn_sb, b1hn_d[:, :].rearrange("p (s c b) -> p s c b", s=SPC, c=NKCH))
        b0f_sb = consts.tile([128, NMCH, SPC, BL], BF16)
        nc.sync.dma_start(b0f_sb, b0f_d[:, :, :].rearrange("p m (s b) -> p m s b", s=SPC))
        b1f_sb = consts.tile([128, NMCH, SPC, BL], BF16)
        nc.sync.dma_start(b1f_sb, b1f_d[:, :, :].rearrange("p m (s b) -> p m s b", s=SPC))
        fcW_sb = consts.tile([128, NKCH], BF16)
        nc.sync.dma_start(fcW_sb, fcWT_d[:, :])
        fcb_sb = consts.tile([BL, 1], F32)
        nc.sync.dma_start(fcb_sb, fcb_d[:, :])
        zeros2 = consts.tile([128, NKCH, BL], BF16)
        nc.vector.memset(zeros2, 0.0)

        def evac(out_ap, psum_ap, bias_ap, bias_bcast_ap):
            """PSUM->SBUF move with bias add, alternating ScalarE/VectorE.
            DVE uses tensor_add with a broadcast-bias constant (TensorScalarPtr
            is rejected by walrus when Tile attaches >1 sync wait)."""
            evac_ctr[0] += 1
            if evac_ctr[0] % 2 == 0:
                nc.scalar.activation(out_ap, psum_ap, AF.Identity, bias=bias_ap)
            else:
                nc.vector.tensor_add(out_ap, psum_ap, bias_bcast_ap)

        def emit_xg_gemm0(xin_w):
            subs = []
            for nch in range(NCH):
                xg_sub = xgp.tile([128, SPC, 8, BL], BF16, tag="xg0")
                nc.vector.tensor_copy(xg_sub[:, :, 4:6, :], b0hn_sb)
                for m in range(NMCH):
                    P = psg.tile([128, SPC, BL], F32, tag="psg")
                    nc.tensor.matmul(
                        P,
                        wih0_sb[:, bass.ts(m, 128)],
                        xin_w[:, bass.ts(nch, 512)],
                        start=True,
                        stop=True,
                    )
                    evac(xg_sub[:, :, m if m < 4 else m + 2, :], P,
                         b0_sb[:, m : m + 1], b0f_sb[:, m, :, :])
                subs.append(xg_sub)
            return subs

        def emit_xg_gemm1(h1win):
            subs = []
            for nch in range(NCH):
                xg_sub = xgp.tile([128, SPC, 8, BL], BF16, tag="xg1")
                nc.vector.tensor_copy(xg_sub[:, :, 4:6, :], b1hn_sb)
                for m in range(NMCH):
                    P = psg.tile([128, SPC, BL], F32, tag="psg")
                    for kc in range(NKCH):
                        nc.tensor.matmul(
                            P,
                            wih1_sb[:, kc, bass.ts(m, 128)],
                            h1win[:, kc, bass.ts(nch, SPC), :],
                            start=(kc == 0),
                            stop=(kc == NKCH - 1),
                        )
                    evac(xg_sub[:, :, m if m < 4 else m + 2, :], P,
                         b1_sb[:, m : m + 1], b1f_sb[:, m, :, :])
                subs.append(xg_sub)
            return subs

        def emit_gru_step(tag, whh_sb, xg_sub, tl, hprev, hout):
            P = psc.tile([128, NMCH, BL], F32, tag="ps" + tag)
            for m in range(NMCH):
                for kc in range(NKCH):
                    nc.tensor.matmul(
                        P[:, m, :],
                        whh_sb[:, kc, bass.ts(m, 128)],
                        hprev[:, kc, :],
                        start=(kc == 0),
                        stop=(kc == NKCH - 1),
                    )
            xg_t = xg_sub[:, tl, :, :]
            # one add covers r/z gate pre-activations AND (hg_n + b_hn)
            a_all = gp.tile([128, 6, BL], BF16, tag=tag + "a_all")
            nc.vector.tensor_add(a_all, P, xg_t[:, 0:6, :])
            rz = gp.tile([128, 4, BL], BF16, tag=tag + "rz")
            nc.scalar.activation(rz, a_all[:, 0:4, :], AF.Sigmoid)
            rh = gp.tile([128, 2, BL], BF16, tag=tag + "rh")
            nc.vector.tensor_mul(rh, a_all[:, 4:6, :], rz[:, 0:2, :])
            a_n = gp.tile([128, 2, BL], BF16, tag=tag + "a_n")
            nc.vector.tensor_add(a_n, rh, xg_t[:, 6:8, :])
            n_sb = gp.tile([128, 2, BL], BF16, tag=tag + "n")
            nc.scalar.activation(n_sb, a_n, AF.Tanh)
            d = gp.tile([128, 2, BL], BF16, tag=tag + "d")
            nc.vector.tensor_sub(d, hprev, n_sb)
            zd = gp.tile([128, 2, BL], BF16, tag=tag + "zd")
            nc.vector.tensor_mul(zd, rz[:, 2:4, :], d)
            nc.vector.tensor_add(hout, zd, n_sb)

        # ---- main pipeline ----
        h1_tail = zeros2[:, :, :]
        h2_prev = zeros2[:, :, :]
        xg1_subs_prev = None
        h1_cur = None
        for w in range(NW + 1):
            if w < NW:
                xin_w = xinp.tile([KIN, T * BL], BF16, tag="xin")
                nc.sync.dma_start(xin_w, xinT_d[:, w * T * BL : (w + 1) * T * BL])
                xg0_subs = emit_xg_gemm0(xin_w)
                h1_cur = h1p.tile([128, NKCH, T, BL], BF16, tag="h1w")
            for t in range(T):
                if w < NW:
                    hprev0 = h1_tail if t == 0 else h1_cur[:, :, t - 1, :]
                    emit_gru_step(
                        "s0", whh0_sb, xg0_subs[t // SPC], t % SPC, hprev0,
                        h1_cur[:, :, t, :],
                    )
                if w > 0:
                    h2_new = h2p.tile([128, NKCH, BL], BF16, tag="h2")
                    emit_gru_step(
                        "s1", whh1_sb, xg1_subs_prev[t // SPC], t % SPC,
                        h2_prev, h2_new,
                    )
                    h2_prev = h2_new
            if w < NW:
                xg1_subs_prev = emit_xg_gemm1(h1_cur)
                h1_tail = h1_cur[:, :, T - 1, :]

        # ---- FC head on the final h2 ----
        Pfc = psg.tile([BL, 1], F32, tag="psg")
        for kc in range(NKCH):
            nc.tensor.matmul(
                Pfc,
                h2_prev[:, kc, :],
                fcW_sb[:, kc : kc + 1],
                start=(kc == 0),
                stop=(kc == NKCH - 1),
            )
        y_sb = gp.tile([BL, 1], F32, tag="y")
        nc.scalar.activation(y_sb, Pfc, AF.Identity, bias=fcb_sb[:, 0:1])
        nc.sync.dma_start(y_d[:, :], y_sb)

    nc.compile()
    return nc


def prep_core_inputs(inputs, core, S=S_FULL):
    """Numpy layout prep for one core's shard (batch rows [32c, 32c+32))."""
    bf = ml_dtypes.bfloat16
    sl = slice(core * BL, (core + 1) * BL)
    x = np.asarray(inputs["x"], np.float32)[sl, :S]  # [BL, S, 4]
    meta = np.asarray(inputs["meta"], np.float32)[sl]  # [BL, 4]
    xin = np.concatenate(
        [x, np.broadcast_to(meta[:, None, :], (BL, S, meta.shape[-1]))], axis=-1
    )  # [BL, S, 8]
    xinT = np.ascontiguousarray(xin.transpose(2, 1, 0)).reshape(KIN, S * BL)

    def whhT(Wname):
        W = np.asarray(inputs[Wname], np.float32)  # [G, H]
        WT = W.T.reshape(NKCH, 128, G).transpose(1, 0, 2)  # [128, NKCH, G]
        return np.ascontiguousarray(WT).astype(bf)

    def bT(b_ih, b_hh):
        # r/z chunks: b_ih + b_hh; n chunks: b_ih only (b_hn goes inside r*(...))
        b = np.asarray(inputs[b_ih], np.float32).copy()
        b[: 2 * H] += np.asarray(inputs[b_hh], np.float32)[: 2 * H]
        return np.ascontiguousarray(b.reshape(NMCH, 128).T).astype(np.float32)

    SPC = 16

    def bfull(b_ih, b_hh):
        b = np.asarray(inputs[b_ih], np.float32).copy()
        b[: 2 * H] += np.asarray(inputs[b_hh], np.float32)[: 2 * H]
        bT = b.reshape(NMCH, 128).T.astype(bf)  # [128, NMCH]
        full = np.broadcast_to(bT[:, :, None, None], (128, NMCH, SPC, BL))
        return np.ascontiguousarray(full).reshape(128, NMCH, SPC * BL)

    def bhn(b_hh):
        b = np.asarray(inputs[b_hh], np.float32)[2 * H :]
        bT = b.reshape(NKCH, 128).T.astype(bf)  # [128, NKCH]
        full = np.broadcast_to(bT[:, None, :, None], (128, SPC, NKCH, BL))
        return np.ascontiguousarray(full).reshape(128, SPC * NKCH * BL)

    wih0T = np.ascontiguousarray(np.asarray(inputs["W_ih0"], np.float32).T).astype(bf)
    fcW = np.asarray(inputs["fc_W"], np.float32).reshape(H)  # [256]
    fcWT = np.ascontiguousarray(fcW.reshape(NKCH, 128).T).astype(bf)
    fcb = np.full((BL, 1), float(np.asarray(inputs["fc_b"]).reshape(-1)[0]), np.float32)

    return {
        "xinT": xinT.astype(bf),
        "wih0T": wih0T,
        "whh0T": whhT("W_hh0"),
        "wih1T": whhT("W_ih1"),
        "whh1T": whhT("W_hh1"),
        "b0T": bT("b_ih0", "b_hh0"),
        "b1T": bT("b_ih1", "b_hh1"),
        "b0hn": bhn("b_hh0"),
        "b1hn": bhn("b_hh1"),
        "b0f": bfull("b_ih0", "b_hh0"),
        "b1f": bfull("b_ih1", "b_hh1"),
        "fcWT": fcWT,
        "fcb": fcb,
    }


_PROGRAM = None


def kernel(**inputs):
    global _PROGRAM
    if _PROGRAM is None:
        _PROGRAM = build_program()
    in_maps = [prep_core_inputs(inputs, c) for c in range(NCORES)]
    res = run_bass_kernel_spmd(_PROGRAM, in_maps, list(range(NCORES))).results
    y = np.concatenate([np.asarray(res[c]["y"], np.float32) for c in range(NCORES)], 0)
    return y.astype(np.float32)
